# revision 22
# baseline (speedup 1.0000x reference)
"""MinLSTM cell (Heinsen-scan reference) as a Bass/Tile kernel for 8 trn2 NeuronCores.

Strategy:
  - Data-parallel over batch N=8: one batch element per core, W/b replicated.
  - Host-side sharding prep: x[n] is transposed to [H_in, L] and cast to fp16 so the
    contraction dim (H_in) lands on SBUF partitions for the PE matmul; W likewise.
  - Device per core:
      pre = W @ x^T  (PE, fp16 inputs, fp32 PSUM), layout [3H, L] channel-major
      sf = sigmoid(pre_f + b_f); si = sigmoid(pre_i + b_i); sh = sigmoid(pre_h + b_h)  (ACT)
      s = sf + si            (DMA CCE accumulate, SBUF->SBUF)
      r = 1/s                (DVE reciprocal_approx_fast)
      f' = sf * r            (DVE)
      htil = max(pre_h + b_h + 0.5, sh)        (DVE STT; exact identity for the
                                                reference g(x)=x>=0? x+0.5 : sigmoid(x))
      w = (f' - 1) * htil    (DVE STT)
      h_t = f'*h_{t-1} - w_t (DVE tensor_tensor_scan, op0=mult op1=subtract, init 1e-6)
    This is the reference's log-space scan rewritten exactly in linear space:
    h_t = f'_t h_{t-1} + (1-f'_t) g(pre_h), a convex combination (stable, all terms > 0).
  - Output [H, L] fp32 per core; host transposes back to (L, H) and stacks.
"""

import os
import sys

import numpy as np

sys.path.insert(0, "/opt/trn_rl_repo")

import concourse.bass as bass  # noqa: E402
import concourse.tile as tile  # noqa: E402
from concourse import bacc, mybir  # noqa: E402

N, L, H_IN, H = 8, 4096, 512, 512
H3 = 3 * H
P = 128
NK = H_IN // P  # 4 k-blocks of the contraction dim
NCT = H // P  # 4 channel tiles
LT = 512  # psum/matmul l-tile (one PSUM bank of fp32)
LH = 2048  # l-granularity of the big DVE ops
NLT = L // LT
NLH = L // LH

F32 = mybir.dt.float32
F16 = mybir.dt.float16
Alu = mybir.AluOpType
Act = mybir.ActivationFunctionType

HX_INIT = 1e-6

_cached_nc = {}


def build_program(L=L, LH=LH):
    key = (L, LH)
    if key in _cached_nc:
        return _cached_nc[key]
    NLH = L // LH

    nc = bacc.Bacc()
    xT_d = nc.dram_tensor("xT", [H_IN, L], F16, kind="ExternalInput")
    wT_d = nc.dram_tensor("wT", [H_IN, H3], F16, kind="ExternalInput")
    bias_d = nc.dram_tensor("bias", [P, 16], F32, kind="ExternalInput")
    out_d = nc.dram_tensor("out", [H, L], F16, kind="ExternalOutput")

    with tile.TileContext(nc) as tc:
        with (
            tc.tile_pool(name="const", bufs=1) as const_pool,
            tc.tile_pool(name="gates", bufs=2) as gates_pool,
            tc.tile_pool(name="sig3", bufs=3) as sig3_pool,
            tc.tile_pool(name="scanbuf", bufs=2) as scan_pool,
            tc.tile_pool(name="psum", bufs=2, space="PSUM") as psum_pool,
        ):
            # Warmup activation with minimal sync deps: absorbs the one-time
            # sigmoid act-table load (walrus rejects table-load + multi-wait
            # on one Activation instruction).
            warm = const_pool.tile([P, 8], F32)
            nc.vector.memset(warm[:], 0.0)
            nc.scalar.activation(warm[:], warm[:], Act.Sigmoid)

            xT_sb = const_pool.tile([P, NK, L], F16)
            wT_sb = const_pool.tile([P, NK, H3], F16)
            bias_sb = const_pool.tile([P, 16], F32)

            nc.sync.dma_start(
                wT_sb[:], wT_d.rearrange("(ki p) o -> p ki o", p=P)
            )
            nc.sync.dma_start(bias_sb[:], bias_d[:])
            # x loaded in L-chunks so the first matmuls start early
            xT_r = xT_d.rearrange("(ki p) l -> p ki l", p=P)
            xoff = 0
            if L >= 4096:
                xchunks = [512, 512, 1024] + [2048] * ((L - 2048) // 2048)
            else:
                xchunks = [512] * (L // 512)
            for xch in xchunks:
                nc.scalar.dma_start(
                    xT_sb[:, :, xoff : xoff + xch],
                    xT_r[:, :, xoff : xoff + xch],
                )
                xoff += xch

            for c in range(NCT):
                # full-L output of the scan for this channel tile
                hv = scan_pool.tile([P, L], F16, tag="hv")

                if L >= 4096:
                    if c == 0:
                        chunks = [512, 1536] + [2048] * ((L - 2048) // 2048)
                    else:
                        chunks = [2048] * (L // 2048)
                else:
                    chunks = [512] * (L // 512)
                ls = 0
                for lh, LHC in enumerate(chunks):
                    sigf = sig3_pool.tile([P, LHC], F32, tag="sigf")
                    sigi = sig3_pool.tile([P, LHC], F32, tag="sigi")
                    htil = gates_pool.tile([P, LHC], F16, tag="htil")
                    shlh = gates_pool.tile([P, LHC], F16, tag="shlh")
                    rl = gates_pool.tile([P, LHC], F16, tag="rl")

                    # One 4-bank PSUM tile per gate; each gate's sigmoid is a
                    # single full-chunk ACT instruction (less ACT overhead).
                    def gate_mms(ps, ocol):
                        for j in range(LHC // LT):
                            xk = slice(ls + j * LT, ls + (j + 1) * LT)
                            jl = slice(j * LT, (j + 1) * LT)
                            for ki in range(NK):
                                nc.tensor.matmul(
                                    ps[:, jl],
                                    wT_sb[:, ki, ocol : ocol + P],
                                    xT_sb[:, ki, xk],
                                    start=ki == 0,
                                    stop=ki == NK - 1,
                                )

                    psH = psum_pool.tile([P, LHC], F32, tag="ps")
                    gate_mms(psH, (2 * NCT + c) * P)
                    nc.scalar.activation(
                        shlh[:], psH[:], Act.Sigmoid,
                        bias=bias_sb[:, 2 * NCT + c : 2 * NCT + c + 1],
                    )
                    # htil = relu(x) + min(sigmoid(x), 0.5), x = psH + b_h
                    # (exact identity for g). relu on ACT; min-vs-const on
                    # DVE tensor_scalar (4x fp16); + as 2x fp16 TT.
                    nc.scalar.activation(
                        rl[:], psH[:], Act.Relu,
                        bias=bias_sb[:, 2 * NCT + c : 2 * NCT + c + 1],
                    )
                    psF = psum_pool.tile([P, LHC], F32, tag="ps")
                    gate_mms(psF, (0 * NCT + c) * P)
                    nc.scalar.activation(
                        sigf[:], psF[:], Act.Sigmoid,
                        bias=bias_sb[:, 0 * NCT + c : 0 * NCT + c + 1],
                    )
                    psI = psum_pool.tile([P, LHC], F32, tag="ps")
                    gate_mms(psI, (1 * NCT + c) * P)
                    nc.scalar.activation(
                        sigi[:], psI[:], Act.Sigmoid,
                        bias=bias_sb[:, 1 * NCT + c : 1 * NCT + c + 1],
                    )

                    mn = gates_pool.tile([P, LHC], F16, tag="mn")
                    nc.vector.tensor_scalar_min(mn[:], shlh[:], 0.5)
                    nc.vector.tensor_tensor(htil[:], rl[:], mn[:], Alu.add)
                    # s = sigf + sigi accumulated into sigi via DMA CCE
                    # (first chunk: on DVE, skipping the CCE latency at startup)
                    if c == 0 and lh == 0:
                        nc.vector.tensor_tensor(sigi[:], sigi[:], sigf[:], Alu.add)
                    else:
                        nc.gpsimd.dma_start(
                            out=sigi[:], in_=sigf[:], accum_op=Alu.add
                        )
                    rcp = gates_pool.tile([P, LHC], F32, tag="rcp")
                    nc.vector.reciprocal_approx_fast(rcp[:], sigi[:])
                    fp = gates_pool.tile([P, LHC], F16, tag="fp")
                    nc.vector.tensor_tensor(fp[:], sigf[:], rcp[:], Alu.mult)
                    # wv = (fp - 1) * htil as 4x tensor_scalar + 2x fp16 TT
                    fpm1 = gates_pool.tile([P, LHC], F16, tag="fpm1")
                    nc.vector.tensor_scalar_add(fpm1[:], fp[:], -1.0)
                    wv = gates_pool.tile([P, LHC], F16, tag="wv")
                    nc.vector.tensor_tensor(wv[:], fpm1[:], htil[:], Alu.mult)
                    init = HX_INIT if lh == 0 else hv[:, ls - 1 : ls]
                    nc.vector.tensor_tensor_scan(
                        hv[:, ls : ls + LHC], fp[:], wv[:], init,
                        Alu.mult, Alu.subtract,
                    )
                    nc.sync.dma_start(
                        out_d[c * P : (c + 1) * P, ls : ls + LHC],
                        hv[:, ls : ls + LHC],
                    )
                    ls += LHC

    nc.compile()
    _cached_nc[key] = nc
    return nc


def _prep_core_inputs(x_n: np.ndarray, wT16: np.ndarray, bias: np.ndarray):
    return {
        "xT": np.ascontiguousarray(x_n.T).astype(np.float16),
        "wT": wT16,
        "bias": bias,
    }


def kernel(x: np.ndarray, W: np.ndarray, b: np.ndarray) -> np.ndarray:
    from concourse.bass_utils import run_bass_kernel_spmd

    nc = build_program()

    wT16 = np.ascontiguousarray(W.T).astype(np.float16)
    b32 = np.asarray(b, dtype=np.float32)
    bias = np.empty((P, 16), dtype=np.float32)
    for j in range(12):
        bias[:, j] = b32[j * P : (j + 1) * P]
    for c in range(NCT):
        bias[:, 12 + c] = b32[2 * H + c * P : 2 * H + (c + 1) * P] + 0.5

    in_maps = [_prep_core_inputs(np.asarray(x[n]), wT16, bias) for n in range(N)]
    res = run_bass_kernel_spmd(nc, in_maps, list(range(N)))

    out = np.empty((N, L, H), dtype=np.float32)
    for n in range(N):
        out[n] = res.results[n]["out"].T.astype(np.float32)
    return out
